# revision 39
# baseline (speedup 1.0000x reference)
"""3-layer GCN (DGL GraphConv, norm='both') on 8 Trainium2 NeuronCores.

Strategy:
  - Nodes are packed into 80 balanced bins (128 slots each) by in-degree
    (greedy least-loaded), 10 bins per core -> 1280 padded rows/core.
  - Edges live with the owner (bin) of their dst node. segment_sum is done
    as one-hot "scatter matmuls" on the TensorEngine: for each dst block,
    agg[128d, D] += S_kt[128e, 128d].T @ msg_kt[128e, D], with S a
    host-built one-hot matrix carrying norm_src[src]*norm_dst[dst].
  - Everything flows in bf16 (PSUM accumulation stays fp32).
  - Layer 1 messages are HOST-EXPANDED (gather indices are host-known), so
    L1 needs no SWDGE descriptors - plain contiguous HWDGE loads spread
    over both HWDGE rings (sync + scalar engines).
  - Activations are exchanged with staged AllGathers in 3 chunks
    (blocks 0-2 / 3-5 / 6-9); a tiny warm-up AllGather at kernel start
    absorbs the first-collective init latency. Each block's edges are
    grouped by the half (blocks 0-5 / 6-9) owning their src row, so
    layers 2/3 run
    their SpMM in two passes: pass A (first-half sources) starts as soon
    as the covering AG chunks land, overlapping the previous layer's
    tail; partial aggregates park in SBUF (bf16) and are re-injected
    into PSUM via an identity matmul.
  - Dense W matmuls run per dst block: PE-transpose agg -> aggT, then
    x = aggT.T @ W with ReLU fused into the PSUM->SBUF copy.
  - Layer 3 computes y3 = x3 @ W3 locally first (padded to 128 wide so
    bf16 gather rows are 256B), AllGathers the small y3, then aggregates:
    A (x W3) == (A x) W3.
"""
import sys
sys.path.insert(0, '/opt/trn_rl_repo')
import numpy as np
import ml_dtypes

BF16 = ml_dtypes.bfloat16
N_CORES = 8
SPL_AG = [0, 6, 10]           # AllGather chunk boundaries (blocks)
SPL_DEP = [0, 6, 10]          # SpMM pass boundaries (blocks)


# ---------------------------------------------------------------- host prep
def _partition_nodes(deg_in, n_nodes, nbins):
    """Greedy balanced-edge binning: nodes (sorted by in-degree desc) go to
    the least-loaded bin with a free slot (capacity 128)."""
    import heapq
    order = np.argsort(-deg_in, kind="stable")
    heap = [(0, b) for b in range(nbins)]
    heapq.heapify(heap)
    bin_of = np.empty(n_nodes, np.int32)
    slot_of = np.empty(n_nodes, np.int32)
    count = np.zeros(nbins, np.int64)
    load = np.zeros(nbins, np.int64)
    for n in order:
        while True:
            l, b = heapq.heappop(heap)
            if count[b] < 128:
                break
            # full bin: drop from heap permanently
        bin_of[n] = b
        slot_of[n] = count[b]
        count[b] += 1
        load[b] += int(deg_in[n])
        heapq.heappush(heap, (l + int(deg_in[n]), b))
    return bin_of, slot_of, load


def _chunks(nkt, ch):
    """Split nkt k-tiles into chunks of at most ch."""
    out = []
    k = 0
    while k < nkt:
        c = min(ch, nkt - k)
        out.append((k, c))
        k += c
    return out


def _prep(h, src, dst, cfg):
    """Build per-core S one-hot tiles, gather indices, and row maps."""
    N, E, NBLK = cfg["N"], cfg["E"], cfg["NBLK"]
    CH = cfg["CH"]
    nbins = N_CORES * NBLK
    deg_out = np.bincount(src, minlength=N)
    deg_in = np.bincount(dst, minlength=N)
    norm_src = np.clip(deg_out, 1, None).astype(np.float32) ** np.float32(-0.5)
    norm_dst = np.clip(deg_in, 1, None).astype(np.float32) ** np.float32(-0.5)
    w = (norm_src[src] * norm_dst[dst]).astype(np.float32)

    bin_of, slot_of, load = _partition_nodes(deg_in, N, nbins)

    # deal bins to cores snake-wise by load to balance core totals
    order = np.argsort(-load, kind="stable")
    core_of_bin = np.empty(nbins, np.int32)
    blk_of_bin = np.empty(nbins, np.int32)
    nextblk = [0] * N_CORES
    for i, b in enumerate(order):
        r = i // N_CORES
        c = (i % N_CORES) if r % 2 == 0 else (N_CORES - 1 - (i % N_CORES))
        core_of_bin[b] = c
        blk_of_bin[b] = nextblk[c]
        nextblk[c] += 1

    RPC = NBLK * 128
    row_of_node = (core_of_bin[bin_of] * RPC + blk_of_bin[bin_of] * 128
                   + slot_of).astype(np.int32)
    # gather-id layout after the staged chunk AllGathers: AG chunk a holds
    # rows [b_a, e_a) of every core, concatenated core-major at offset 8*b_a
    spa = np.array(SPL_AG) * 128
    _c = row_of_node // RPC
    _r = row_of_node % RPC
    _a = np.searchsorted(spa, _r, side="right") - 1
    gid_of_node = (N_CORES * spa[_a] + _c * (spa[_a + 1] - spa[_a])
                   + _r - spa[_a]).astype(np.int32)
    # SpMM pass of a src node + pass-relative gid
    spd = np.array(SPL_DEP) * 128
    pass_of_node = (np.searchsorted(spd, _r, side="right") - 1).astype(np.int32)
    relgid_of_node = (gid_of_node
                      - N_CORES * spd[pass_of_node]).astype(np.int32)

    # group edges by (dst bin, src pass)
    epass = pass_of_node[src]
    ebin = bin_of[dst]
    key = ebin * 2 + epass
    eorder = np.argsort(key, kind="stable")
    counts = np.bincount(key, minlength=nbins * 2).reshape(nbins, 2)
    kts0 = int(-(-counts[:, 0].max() // 128))
    kts1 = int(-(-counts[:, 1].max() // 128))
    kt_blk = kts0 + kts1
    kt_tot = NBLK * kt_blk

    slots = np.zeros((N_CORES, kt_tot * 128), np.int32)       # absolute src id
    idx23 = np.zeros((N_CORES, kt_tot * 128), np.int16)       # pass-rel gid
    S = np.zeros((N_CORES, 128, kt_tot, 128), np.float32)
    bounds = np.concatenate([[0], np.cumsum(counts.reshape(-1))])
    for b in range(nbins):
        c, blk = int(core_of_bin[b]), int(blk_of_bin[b])
        for q in range(2):
            kbase = blk * kt_blk + (0 if q == 0 else kts0)
            es = eorder[bounds[b * 2 + q]:bounds[b * 2 + q + 1]]
            p = np.arange(len(es))
            kt = kbase + p // 128
            esl = p % 128
            gpos = kt * 128 + esl
            slots[c, gpos] = src[es]
            idx23[c, gpos] = relgid_of_node[src[es]].astype(np.int16)
            S[c, esl, kt, slot_of[dst[es]]] = w[es]

    def wrap(ix):  # -> [128, kt_tot*8] wrapped for the 8 Q7 cores
        return np.tile(ix.reshape(-1, 16).T, (8, 1)).copy()

    idx23_w = np.stack([wrap(idx23[c]) for c in range(N_CORES)])

    return dict(S=S.astype(BF16), slots=slots, idx23=idx23_w,
                row_of_node=row_of_node, kts0=kts0, kts1=kts1,
                kt_blk=kt_blk, kt_tot=kt_tot)


# ---------------------------------------------------------------- device prog
def _build(cfg, kts0, kts1, use_bias):
    import concourse.bacc as bacc
    import concourse.mybir as mybir
    import concourse.tile as tile
    from concourse.library_config import mlp

    f32 = mybir.dt.float32
    bf16 = mybir.dt.bfloat16
    i16 = mybir.dt.int16
    RELU = mybir.ActivationFunctionType.Relu
    COPY = mybir.ActivationFunctionType.Copy

    N, D, C, NBLK = cfg["N"], cfg["D"], cfg["C"], cfg["NBLK"]
    CP = 128                    # layer-3 padded width
    RPC = NBLK * 128
    NPAD = N_CORES * RPC
    KT = kts0 + kts1
    KT_TOT = NBLK * KT
    CH = cfg["CH"]
    CH1 = cfg["CH1"]            # k-tiles per L1 contiguous-load chunk
    KD = D // 128               # dense contraction k-tiles
    ND = 512 if D % 512 == 0 else D
    NT = D // ND                # dense n-tiles
    TPW = min(1024, D)          # transposes packed per tps tile (bf16: 1 bank)
    TPG = TPW // 128
    chunks = ([(k, c, 0) for k, c in _chunks(kts0, CH)]
              + [(kts0 + k, c, 1) for k, c in _chunks(kts1, CH)])
    spd = [N_CORES * b * 128 for b in SPL_DEP]   # pass row bounds in ag_out

    nc = bacc.Bacc("TRN2", target_bir_lowering=False, debug=False,
                   num_devices=N_CORES, num_swdge_queues=4,
                   dynamic_dma_scratch_size=16384)

    msg1_h = nc.dram_tensor("msg1", [128, KT_TOT, D], bf16,
                            kind="ExternalInput")
    sker = nc.dram_tensor("sker", [128, KT_TOT, 128], bf16,
                          kind="ExternalInput")
    idx23_h = nc.dram_tensor("idx23", [128, KT_TOT * 8], i16, kind="ExternalInput")
    w12_h = nc.dram_tensor("w12", [2, 128, KD, D], bf16, kind="ExternalInput")
    w3_h = nc.dram_tensor("w3", [128, KD, CP], bf16, kind="ExternalInput")
    ident_h = nc.dram_tensor("ident", [128, 128], bf16, kind="ExternalInput")
    bias_h = nc.dram_tensor("biases", [1, 2 * D + CP + 512], bf16,
                            kind="ExternalInput")
    out_h = nc.dram_tensor("out", [RPC, C], f32, kind="ExternalOutput")

    ag_in = nc.dram_tensor("ag_in", [RPC, D], bf16, kind="Internal")
    ag_out = nc.dram_tensor("ag_out", [NPAD, D], bf16, kind="Internal",
                            addr_space="Shared")
    ag3_in = nc.dram_tensor("ag3_in", [RPC, CP], bf16, kind="Internal")
    ag3_out = nc.dram_tensor("ag3_out", [NPAD, CP], bf16, kind="Internal",
                             addr_space="Shared")
    agw_in = nc.dram_tensor("agw_in", [16, 16], bf16, kind="Internal")
    agw_out = nc.dram_tensor("agw_out", [128, 16], bf16, kind="Internal",
                             addr_space="Shared")

    with tile.TileContext(nc) as tc:
        nc.gpsimd.load_library(mlp)
        with (
            tc.tile_pool(name="const", bufs=1) as cp,
            tc.tile_pool(name="msg1", bufs=5) as mp1,
            tc.tile_pool(name="msg", bufs=4) as mp,
            tc.tile_pool(name="msg3", bufs=4) as mp3,
            tc.tile_pool(name="part", bufs=1) as pp,
            tc.tile_pool(name="work", bufs=2) as wp,
            tc.tile_pool(name="aggps", bufs=2, space="PSUM") as aps,
            tc.tile_pool(name="densps", bufs=2, space="PSUM") as dps,
            tc.tile_pool(name="tpsps", bufs=2, space="PSUM") as tps,
        ):
            dctr = [0]

            def dma(dst, src):
                """Alternate loads across the two HWDGE rings."""
                eng = nc.sync if dctr[0] % 2 == 0 else nc.scalar
                dctr[0] += 1
                eng.dma_start(dst, src)

            ident_t = cp.tile([128, 128], bf16, tag="ident")
            nc.sync.dma_start(ident_t[:], ident_h[:])
            w_t = cp.tile([128, KD, D], bf16, tag="w")
            nc.scalar.dma_start(w_t[:], w12_h[0])
            idx23_t = cp.tile([128, KT_TOT * 8], i16, tag="idx23")
            nc.scalar.dma_start(idx23_t[:], idx23_h[:])
            w3_t = cp.tile([128, KD, CP], bf16, tag="w3")
            nc.scalar.dma_start(w3_t[:], w3_h[:])
            if use_bias:
                brow_t = cp.tile([1, 2 * D + CP + 512], bf16, tag="brow")
                nc.scalar.dma_start(brow_t[:], bias_h[:])
                ones_t = brow_t[:, 2 * D + CP:2 * D + CP + 512]
            s_blk = [None] * NBLK

            def load_s(b):
                sb = cp.tile([128, KT, 128], bf16, tag=f"s{b}")
                dma(sb[:], sker[:, b * KT:(b + 1) * KT, :])
                s_blk[b] = sb

            qctr = [0]

            def sp_mms(agg, b, msg, k0, ch, width, first, last):
                """One-hot scatter matmuls for ch k-tiles into agg."""
                nspl = max(1, width // 512)
                for k in range(ch):
                    for n in range(nspl):
                        w0 = n * (width // nspl)
                        w1 = (n + 1) * (width // nspl)
                        nc.tensor.matmul(
                            agg[:, w0:w1], s_blk[b][:, k0 + k, :],
                            msg[:, k, w0:w1],
                            start=first and k == 0,
                            stop=last and k == ch - 1)

            def spmm_l1(b):
                """agg for dst block b from host-expanded messages."""
                agg = aps.tile([128, D], f32, tag="aggps")
                lds = _chunks(KT, CH1)
                for i, (k0, ch) in enumerate(lds):
                    msg = mp1.tile([128, CH1, D], bf16, tag="m1")
                    dma(msg[:, :ch, :],
                        msg1_h[:, b * KT + k0:b * KT + k0 + ch, :])
                    sp_mms(agg, b, msg, k0, ch, D,
                           first=(i == 0), last=(i == len(lds) - 1))
                return agg

            def spmm_pass(b, q, src_h, width, msg_pool, tag, partial=None,
                          close=True):
                """Gather + scatter-matmul the k-tiles of pass q, block b."""
                agg = aps.tile([128, width], f32, tag="aggps")
                mychunks = [cc for cc in chunks if cc[2] == q]
                if partial is not None:
                    nspl = max(1, width // 512)
                    for n in range(nspl):
                        w0, w1 = n * (width // nspl), (n + 1) * (width // nspl)
                        nc.tensor.matmul(agg[:, w0:w1], ident_t[:],
                                         partial[:, w0:w1],
                                         start=True, stop=False)
                for i, (k0, ch, _) in enumerate(mychunks):
                    msg = msg_pool.tile([128, CH, width], bf16, tag=tag)
                    col0 = (b * KT + k0) * 8
                    qq = qctr[0] % 4
                    qctr[0] += 1
                    nc.gpsimd.dma_gather(
                        msg[:, :ch, :], src_h[spd[q]:spd[q + 1]],
                        idx23_t[:, col0:col0 + ch * 8],
                        ch * 128, ch * 128, width, queue_num=qq)
                    sp_mms(agg, b, msg, k0, ch, width,
                           first=(i == 0 and partial is None),
                           last=(i == len(mychunks) - 1 and close))
                return agg

            def transpose_to(dst_t, src_sb):
                """dst_t[128, KD, 128] (bf16) = src_sb[128, D] transposed."""
                for g in range(KD // TPG):
                    tp = tps.tile([128, TPW], bf16, tag="tp")
                    for j in range(TPG):
                        col = (g * TPG + j) * 128
                        nc.tensor.transpose(
                            tp[:, j * 128:(j + 1) * 128],
                            src_sb[:, col:col + 128], ident_t[:])
                    nc.vector.tensor_copy(
                        dst_t[:, g * TPG:(g + 1) * TPG, :].rearrange(
                            "p a b -> p (a b)"), tp[:])

            def dense_block(aggT_t, out_sb, bias_off, relu):
                """out_sb[128, D] = act(aggT.T @ W + b)."""
                for n in range(NT):
                    dp = dps.tile([128, ND], f32, tag="dp")
                    for k in range(KD):
                        nc.tensor.matmul(
                            dp[:], aggT_t[:, k, :], w_t[:, k, n * ND:(n + 1) * ND],
                            start=(k == 0), stop=(k == KD - 1 and not use_bias))
                    if use_bias:
                        nc.tensor.matmul(
                            dp[:], ones_t[:, :ND],
                            brow_t[:, bias_off + n * ND:bias_off + (n + 1) * ND],
                            start=False, stop=True)
                    nc.scalar.activation(out_sb[:, n * ND:(n + 1) * ND], dp[:],
                                         RELU if relu else COPY)

            ag_done = {0: set(), 1: set()}
            ag_fired = {0: set(), 1: set()}

            def stage_ag(layer, b, src_dram, dst_dram):
                """Fire any AllGather chunk whose blocks are all finished."""
                ag_done[layer].add(b)
                for a in range(len(SPL_AG) - 1):
                    blks = set(range(SPL_AG[a], SPL_AG[a + 1]))
                    if a in ag_fired[layer] or not blks <= ag_done[layer]:
                        continue
                    ag_fired[layer].add(a)
                    r0, r1 = SPL_AG[a] * 128, SPL_AG[a + 1] * 128
                    nc.gpsimd.collective_compute(
                        "AllGather", mybir.AluOpType.bypass,
                        ins=[src_dram[r0:r1, :]],
                        outs=[dst_dram[N_CORES * r0:N_CORES * r1, :]],
                        replica_groups=[list(range(N_CORES))])

            def finish_block(layer, b, agg):
                """Dense tail for a completed aggregate of layer 1/2."""
                agg_sb = wp.tile([128, D], bf16, tag="aggsb")
                nc.scalar.activation(agg_sb[:], agg[:], COPY)
                aggT_t = wp.tile([128, KD, 128], bf16, tag="aggT")
                transpose_to(aggT_t, agg_sb)
                x_sb = wp.tile([128, D], bf16, tag="x")
                dense_block(aggT_t, x_sb, layer * D, relu=True)
                if layer == 0:
                    # SWDGE write: keeps the HWDGE load rings free-flowing
                    nc.gpsimd.dma_start(ag_in[b * 128:(b + 1) * 128, :],
                                        x_sb[:])
                    stage_ag(0, b, ag_in, ag_out)
                else:
                    # y3 = x3 @ W3 (padded to CP wide) for this block
                    x3T_t = wp.tile([128, KD, 128], bf16, tag="x3T")
                    transpose_to(x3T_t, x_sb)
                    yp = dps.tile([128, CP], f32, tag="dp")
                    for k in range(KD):
                        nc.tensor.matmul(yp[:], x3T_t[:, k, :], w3_t[:, k, :],
                                         start=(k == 0), stop=(k == KD - 1))
                    y_sb = wp.tile([128, CP], bf16, tag="y")
                    nc.scalar.activation(y_sb[:], yp[:], COPY)
                    nc.sync.dma_start(ag3_in[b * 128:(b + 1) * 128, :], y_sb[:])
                    stage_ag(1, b, ag3_in, ag3_out)

            # warm-up collective: absorbs the first-call ncfw/comm init
            # latency while layer 1 computes
            nc.gpsimd.collective_compute(
                "AllGather", mybir.AluOpType.bypass,
                ins=[agw_in[:]], outs=[agw_out[:]],
                replica_groups=[list(range(N_CORES))])

            # ---------------- layer 1 (host-expanded messages, no gathers)
            # 1-block software pipeline: block b+1's scatter-matmuls are
            # queued ahead of block b's transpose/dense so the PE never
            # stalls on the PSUM->SBUF copies.
            ORD = list(range(SPL_DEP[1], NBLK)) + list(range(SPL_DEP[1]))
            prev = prev_b = None
            for b in ORD:
                load_s(b)
                agg = spmm_l1(b)
                if prev is not None:
                    finish_block(0, prev_b, prev)
                prev, prev_b = agg, b
            finish_block(0, prev_b, prev)

            nc.scalar.dma_start(w_t[:], w12_h[1])

            # ---------------- layer 2 pass A: second-half sources (their AG
            # chunk fires first) -> partial aggregates, overlapping L1's tail
            part_t = pp.tile([128, NBLK, D], bf16, tag="pA")
            for b in range(NBLK):
                agg = spmm_pass(b, 1, ag_out, D, mp, "m")
                nc.scalar.activation(part_t[:, b, :], agg[:], COPY)

            # ---------------- layer 2 pass B: first-half + partial -> dense
            prev = prev_b = None
            for b in ORD:
                agg = spmm_pass(b, 0, ag_out, D, mp, "m",
                                partial=part_t[:, b, :])
                if prev is not None:
                    finish_block(1, prev_b, prev)
                prev, prev_b = agg, b
            finish_block(1, prev_b, prev)

            # ---------------- layer 3: out = A y3 (+ b3), two passes
            part3_t = pp.tile([128, NBLK, CP], bf16, tag="pA3")
            for b in range(NBLK):
                agg = spmm_pass(b, 1, ag3_out, CP, mp3, "m3")
                nc.scalar.activation(part3_t[:, b, :], agg[:], COPY)
            for b in range(NBLK):
                agg3 = spmm_pass(b, 0, ag3_out, CP, mp3, "m3",
                                 partial=part3_t[:, b, :],
                                 close=not use_bias)
                if use_bias:
                    nc.tensor.matmul(agg3[:], ones_t[:, :CP],
                                     brow_t[:, 2 * D:2 * D + CP],
                                     start=False, stop=True)
                o_sb = wp.tile([128, C], f32, tag="o")
                nc.scalar.activation(o_sb[:], agg3[:, :C], COPY)
                nc.sync.dma_start(out_h[b * 128:(b + 1) * 128, :], o_sb[:])

    nc.compile()
    return nc


_CACHE = {}


def _get_prog(cfg, kts0, kts1, use_bias):
    key = (cfg["N"], cfg["D"], kts0, kts1, use_bias, cfg["CH"], cfg["CH1"])
    if key not in _CACHE:
        _CACHE[key] = _build(cfg, kts0, kts1, use_bias)
    return _CACHE[key]


# ---------------------------------------------------------------- entry point
CFG_FULL = dict(N=10000, E=160000, D=1024, C=64, NBLK=10, CH=4, CH1=4)


def _make_inmaps(ins, pp, cfg):
    """Per-core input dicts (host arrays in bf16)."""
    D, C, KD, CP = cfg["D"], cfg["C"], cfg["D"] // 128, 128
    h16 = np.asarray(ins["h"], np.float32).astype(BF16)
    kt_tot = pp["kt_tot"]
    w12 = np.stack([
        np.asarray(ins["W1"], np.float32).reshape(KD, 128, D).transpose(1, 0, 2),
        np.asarray(ins["W2"], np.float32).reshape(KD, 128, D).transpose(1, 0, 2),
    ]).astype(BF16)
    w3p = np.zeros((cfg["D"], CP), np.float32)
    w3p[:, :C] = np.asarray(ins["W3"], np.float32)
    w3 = w3p.reshape(KD, 128, CP).transpose(1, 0, 2).astype(BF16)
    biases = np.concatenate([
        np.asarray(ins["b1"], np.float32),
        np.asarray(ins["b2"], np.float32),
        np.asarray(ins["b3"], np.float32), np.zeros(CP - C, np.float32),
        np.ones(512, np.float32)]).astype(BF16)[None, :]
    ident = np.eye(128, dtype=BF16)
    maps = []
    for c in range(N_CORES):
        # host-side edge expansion of layer-1 messages:
        # msg1[p, kt, :] = h[src_of_slot[kt*128 + p]]
        msg1 = h16[pp["slots"][c]].reshape(kt_tot, 128, D).transpose(1, 0, 2)
        maps.append(dict(
            msg1=np.ascontiguousarray(msg1),
            sker=np.ascontiguousarray(pp["S"][c]),
            idx23=pp["idx23"][c],
            w12=w12, w3=w3, ident=ident, biases=biases))
    return maps


def kernel(h, src, dst, W1, b1, W2, b2, W3, b3, cfg=CFG_FULL):
    from concourse.bass_utils import run_bass_kernel_spmd

    h = np.asarray(h, np.float32)
    src = np.asarray(src, np.int32)
    dst = np.asarray(dst, np.int32)
    N, C = cfg["N"], cfg["C"]

    pp = _prep(h, src, dst, cfg)
    use_bias = bool(np.any(b1) or np.any(b2) or np.any(b3))
    nc = _get_prog(cfg, pp["kts0"], pp["kts1"], use_bias)

    ins = dict(h=h, W1=W1, b1=b1, W2=W2, b2=b2, W3=W3, b3=b3)
    in_maps = _make_inmaps(ins, pp, cfg)
    res = run_bass_kernel_spmd(nc, in_maps, core_ids=list(range(N_CORES)))

    out = np.zeros((N, C), np.float32)
    rows = pp["row_of_node"]
    allout = np.concatenate([res.results[c]["out"] for c in range(N_CORES)],
                            axis=0)
    out[:, :] = allout[rows]
    return out


# revision 40
# speedup vs baseline: 1.0095x; 1.0095x over previous
"""3-layer GCN (DGL GraphConv, norm='both') on 8 Trainium2 NeuronCores.

Strategy:
  - Nodes are packed into 80 balanced bins (128 slots each) by in-degree
    (greedy least-loaded), 10 bins per core -> 1280 padded rows/core.
  - Edges live with the owner (bin) of their dst node. segment_sum is done
    as one-hot "scatter matmuls" on the TensorEngine: for each dst block,
    agg[128d, D] += S_kt[128e, 128d].T @ msg_kt[128e, D], with S a
    host-built one-hot matrix carrying norm_src[src]*norm_dst[dst].
  - Everything flows in bf16 (PSUM accumulation stays fp32).
  - Layer 1 messages are HOST-EXPANDED (gather indices are host-known), so
    L1 needs no SWDGE descriptors - plain contiguous HWDGE loads spread
    over both HWDGE rings (sync + scalar engines).
  - Activations are exchanged with staged AllGathers in 3 chunks
    (blocks 0-2 / 3-5 / 6-9); a tiny warm-up AllGather at kernel start
    absorbs the first-collective init latency. Each block's edges are
    grouped by the half (blocks 0-5 / 6-9) owning their src row, so
    layers 2/3 run
    their SpMM in two passes: pass A (first-half sources) starts as soon
    as the covering AG chunks land, overlapping the previous layer's
    tail; partial aggregates park in SBUF (bf16) and are re-injected
    into PSUM via an identity matmul.
  - Dense W matmuls run per dst block: PE-transpose agg -> aggT, then
    x = aggT.T @ W with ReLU fused into the PSUM->SBUF copy.
  - Layer 3 computes y3 = x3 @ W3 locally first (padded to 128 wide so
    bf16 gather rows are 256B), AllGathers the small y3, then aggregates:
    A (x W3) == (A x) W3.
"""
import sys
sys.path.insert(0, '/opt/trn_rl_repo')
import numpy as np
import ml_dtypes

BF16 = ml_dtypes.bfloat16
N_CORES = 8
SPL_AG = [0, 3, 6, 10]        # AllGather chunk boundaries (blocks)
SPL_DEP = [0, 6, 10]          # SpMM pass boundaries (blocks)


# ---------------------------------------------------------------- host prep
def _partition_nodes(deg_in, n_nodes, nbins):
    """Greedy balanced-edge binning: nodes (sorted by in-degree desc) go to
    the least-loaded bin with a free slot (capacity 128)."""
    import heapq
    order = np.argsort(-deg_in, kind="stable")
    heap = [(0, b) for b in range(nbins)]
    heapq.heapify(heap)
    bin_of = np.empty(n_nodes, np.int32)
    slot_of = np.empty(n_nodes, np.int32)
    count = np.zeros(nbins, np.int64)
    load = np.zeros(nbins, np.int64)
    for n in order:
        while True:
            l, b = heapq.heappop(heap)
            if count[b] < 128:
                break
            # full bin: drop from heap permanently
        bin_of[n] = b
        slot_of[n] = count[b]
        count[b] += 1
        load[b] += int(deg_in[n])
        heapq.heappush(heap, (l + int(deg_in[n]), b))
    return bin_of, slot_of, load


def _chunks(nkt, ch):
    """Split nkt k-tiles into chunks of at most ch."""
    out = []
    k = 0
    while k < nkt:
        c = min(ch, nkt - k)
        out.append((k, c))
        k += c
    return out


def _prep(h, src, dst, cfg):
    """Build per-core S one-hot tiles, gather indices, and row maps."""
    N, E, NBLK = cfg["N"], cfg["E"], cfg["NBLK"]
    CH = cfg["CH"]
    nbins = N_CORES * NBLK
    deg_out = np.bincount(src, minlength=N)
    deg_in = np.bincount(dst, minlength=N)
    norm_src = np.clip(deg_out, 1, None).astype(np.float32) ** np.float32(-0.5)
    norm_dst = np.clip(deg_in, 1, None).astype(np.float32) ** np.float32(-0.5)
    w = (norm_src[src] * norm_dst[dst]).astype(np.float32)

    bin_of, slot_of, load = _partition_nodes(deg_in, N, nbins)

    # deal bins to cores snake-wise by load to balance core totals
    order = np.argsort(-load, kind="stable")
    core_of_bin = np.empty(nbins, np.int32)
    blk_of_bin = np.empty(nbins, np.int32)
    nextblk = [0] * N_CORES
    for i, b in enumerate(order):
        r = i // N_CORES
        c = (i % N_CORES) if r % 2 == 0 else (N_CORES - 1 - (i % N_CORES))
        core_of_bin[b] = c
        blk_of_bin[b] = nextblk[c]
        nextblk[c] += 1

    RPC = NBLK * 128
    row_of_node = (core_of_bin[bin_of] * RPC + blk_of_bin[bin_of] * 128
                   + slot_of).astype(np.int32)
    # gather-id layout after the staged chunk AllGathers: AG chunk a holds
    # rows [b_a, e_a) of every core, concatenated core-major at offset 8*b_a
    spa = np.array(SPL_AG) * 128
    _c = row_of_node // RPC
    _r = row_of_node % RPC
    _a = np.searchsorted(spa, _r, side="right") - 1
    gid_of_node = (N_CORES * spa[_a] + _c * (spa[_a + 1] - spa[_a])
                   + _r - spa[_a]).astype(np.int32)
    # SpMM pass of a src node + pass-relative gid
    spd = np.array(SPL_DEP) * 128
    pass_of_node = (np.searchsorted(spd, _r, side="right") - 1).astype(np.int32)
    relgid_of_node = (gid_of_node
                      - N_CORES * spd[pass_of_node]).astype(np.int32)

    # group edges by (dst bin, src pass)
    epass = pass_of_node[src]
    ebin = bin_of[dst]
    key = ebin * 2 + epass
    eorder = np.argsort(key, kind="stable")
    counts = np.bincount(key, minlength=nbins * 2).reshape(nbins, 2)
    kts0 = int(-(-counts[:, 0].max() // 128))
    kts1 = int(-(-counts[:, 1].max() // 128))
    kt_blk = kts0 + kts1
    kt_tot = NBLK * kt_blk

    slots = np.zeros((N_CORES, kt_tot * 128), np.int32)       # absolute src id
    idx23 = np.zeros((N_CORES, kt_tot * 128), np.int16)       # pass-rel gid
    S = np.zeros((N_CORES, 128, kt_tot, 128), np.float32)
    bounds = np.concatenate([[0], np.cumsum(counts.reshape(-1))])
    for b in range(nbins):
        c, blk = int(core_of_bin[b]), int(blk_of_bin[b])
        for q in range(2):
            kbase = blk * kt_blk + (0 if q == 0 else kts0)
            es = eorder[bounds[b * 2 + q]:bounds[b * 2 + q + 1]]
            p = np.arange(len(es))
            kt = kbase + p // 128
            esl = p % 128
            gpos = kt * 128 + esl
            slots[c, gpos] = src[es]
            idx23[c, gpos] = relgid_of_node[src[es]].astype(np.int16)
            S[c, esl, kt, slot_of[dst[es]]] = w[es]

    def wrap(ix):  # -> [128, kt_tot*8] wrapped for the 8 Q7 cores
        return np.tile(ix.reshape(-1, 16).T, (8, 1)).copy()

    idx23_w = np.stack([wrap(idx23[c]) for c in range(N_CORES)])

    return dict(S=S.astype(BF16), slots=slots, idx23=idx23_w,
                row_of_node=row_of_node, kts0=kts0, kts1=kts1,
                kt_blk=kt_blk, kt_tot=kt_tot)


# ---------------------------------------------------------------- device prog
def _build(cfg, kts0, kts1, use_bias):
    import concourse.bacc as bacc
    import concourse.mybir as mybir
    import concourse.tile as tile
    from concourse.library_config import mlp

    f32 = mybir.dt.float32
    bf16 = mybir.dt.bfloat16
    i16 = mybir.dt.int16
    RELU = mybir.ActivationFunctionType.Relu
    COPY = mybir.ActivationFunctionType.Copy

    N, D, C, NBLK = cfg["N"], cfg["D"], cfg["C"], cfg["NBLK"]
    CP = 128                    # layer-3 padded width
    RPC = NBLK * 128
    NPAD = N_CORES * RPC
    KT = kts0 + kts1
    KT_TOT = NBLK * KT
    CH = cfg["CH"]
    CH1 = cfg["CH1"]            # k-tiles per L1 contiguous-load chunk
    KD = D // 128               # dense contraction k-tiles
    ND = 512 if D % 512 == 0 else D
    NT = D // ND                # dense n-tiles
    TPW = min(1024, D)          # transposes packed per tps tile (bf16: 1 bank)
    TPG = TPW // 128
    chunks = ([(k, c, 0) for k, c in _chunks(kts0, CH)]
              + [(kts0 + k, c, 1) for k, c in _chunks(kts1, CH)])
    spd = [N_CORES * b * 128 for b in SPL_DEP]   # pass row bounds in ag_out

    nc = bacc.Bacc("TRN2", target_bir_lowering=False, debug=False,
                   num_devices=N_CORES, num_swdge_queues=4,
                   dynamic_dma_scratch_size=16384)

    msg1_h = nc.dram_tensor("msg1", [128, KT_TOT, D], bf16,
                            kind="ExternalInput")
    sker = nc.dram_tensor("sker", [128, KT_TOT, 128], bf16,
                          kind="ExternalInput")
    idx23_h = nc.dram_tensor("idx23", [128, KT_TOT * 8], i16, kind="ExternalInput")
    w12_h = nc.dram_tensor("w12", [2, 128, KD, D], bf16, kind="ExternalInput")
    w3_h = nc.dram_tensor("w3", [128, KD, CP], bf16, kind="ExternalInput")
    ident_h = nc.dram_tensor("ident", [128, 128], bf16, kind="ExternalInput")
    bias_h = nc.dram_tensor("biases", [1, 2 * D + CP + 512], bf16,
                            kind="ExternalInput")
    out_h = nc.dram_tensor("out", [RPC, C], f32, kind="ExternalOutput")

    ag_in = nc.dram_tensor("ag_in", [RPC, D], bf16, kind="Internal")
    ag_out = nc.dram_tensor("ag_out", [NPAD, D], bf16, kind="Internal",
                            addr_space="Shared")
    ag3_in = nc.dram_tensor("ag3_in", [RPC, CP], bf16, kind="Internal")
    ag3_out = nc.dram_tensor("ag3_out", [NPAD, CP], bf16, kind="Internal",
                             addr_space="Shared")
    agw_in = nc.dram_tensor("agw_in", [16, 16], bf16, kind="Internal")
    agw_out = nc.dram_tensor("agw_out", [128, 16], bf16, kind="Internal",
                             addr_space="Shared")

    with tile.TileContext(nc) as tc:
        nc.gpsimd.load_library(mlp)
        with (
            tc.tile_pool(name="const", bufs=1) as cp,
            tc.tile_pool(name="msg1", bufs=4) as mp1,
            tc.tile_pool(name="msg", bufs=5) as mp,
            tc.tile_pool(name="msg3", bufs=4) as mp3,
            tc.tile_pool(name="part", bufs=1) as pp,
            tc.tile_pool(name="work", bufs=2) as wp,
            tc.tile_pool(name="aggps", bufs=2, space="PSUM") as aps,
            tc.tile_pool(name="densps", bufs=2, space="PSUM") as dps,
            tc.tile_pool(name="tpsps", bufs=2, space="PSUM") as tps,
        ):
            dctr = [0]

            def dma(dst, src):
                """Alternate loads across the two HWDGE rings."""
                eng = nc.sync if dctr[0] % 2 == 0 else nc.scalar
                dctr[0] += 1
                eng.dma_start(dst, src)

            ident_t = cp.tile([128, 128], bf16, tag="ident")
            nc.sync.dma_start(ident_t[:], ident_h[:])
            w_t = cp.tile([128, KD, D], bf16, tag="w")
            nc.scalar.dma_start(w_t[:], w12_h[0])
            idx23_t = cp.tile([128, KT_TOT * 8], i16, tag="idx23")
            nc.scalar.dma_start(idx23_t[:], idx23_h[:])
            w3_t = cp.tile([128, KD, CP], bf16, tag="w3")
            nc.scalar.dma_start(w3_t[:], w3_h[:])
            if use_bias:
                brow_t = cp.tile([1, 2 * D + CP + 512], bf16, tag="brow")
                nc.scalar.dma_start(brow_t[:], bias_h[:])
                ones_t = brow_t[:, 2 * D + CP:2 * D + CP + 512]
            s_blk = [None] * NBLK

            def load_s(b):
                sb = cp.tile([128, KT, 128], bf16, tag=f"s{b}")
                dma(sb[:], sker[:, b * KT:(b + 1) * KT, :])
                s_blk[b] = sb

            qctr = [0]

            def sp_mms(agg, b, msg, k0, ch, width, first, last):
                """One-hot scatter matmuls for ch k-tiles into agg."""
                nspl = max(1, width // 512)
                for k in range(ch):
                    for n in range(nspl):
                        w0 = n * (width // nspl)
                        w1 = (n + 1) * (width // nspl)
                        nc.tensor.matmul(
                            agg[:, w0:w1], s_blk[b][:, k0 + k, :],
                            msg[:, k, w0:w1],
                            start=first and k == 0,
                            stop=last and k == ch - 1)

            def spmm_l1(b):
                """agg for dst block b from host-expanded messages."""
                agg = aps.tile([128, D], f32, tag="aggps")
                lds = _chunks(KT, CH1)
                for i, (k0, ch) in enumerate(lds):
                    msg = mp1.tile([128, CH1, D], bf16, tag="m1")
                    dma(msg[:, :ch, :],
                        msg1_h[:, b * KT + k0:b * KT + k0 + ch, :])
                    sp_mms(agg, b, msg, k0, ch, D,
                           first=(i == 0), last=(i == len(lds) - 1))
                return agg

            def spmm_pass(b, q, src_h, width, msg_pool, tag, partial=None,
                          close=True):
                """Gather + scatter-matmul the k-tiles of pass q, block b."""
                agg = aps.tile([128, width], f32, tag="aggps")
                mychunks = [cc for cc in chunks if cc[2] == q]
                if partial is not None:
                    nspl = max(1, width // 512)
                    for n in range(nspl):
                        w0, w1 = n * (width // nspl), (n + 1) * (width // nspl)
                        nc.tensor.matmul(agg[:, w0:w1], ident_t[:],
                                         partial[:, w0:w1],
                                         start=True, stop=False)
                for i, (k0, ch, _) in enumerate(mychunks):
                    msg = msg_pool.tile([128, CH, width], bf16, tag=tag)
                    col0 = (b * KT + k0) * 8
                    qq = qctr[0] % 4
                    qctr[0] += 1
                    nc.gpsimd.dma_gather(
                        msg[:, :ch, :], src_h[spd[q]:spd[q + 1]],
                        idx23_t[:, col0:col0 + ch * 8],
                        ch * 128, ch * 128, width, queue_num=qq)
                    sp_mms(agg, b, msg, k0, ch, width,
                           first=(i == 0 and partial is None),
                           last=(i == len(mychunks) - 1 and close))
                return agg

            def transpose_to(dst_t, src_sb):
                """dst_t[128, KD, 128] (bf16) = src_sb[128, D] transposed."""
                for g in range(KD // TPG):
                    tp = tps.tile([128, TPW], bf16, tag="tp")
                    for j in range(TPG):
                        col = (g * TPG + j) * 128
                        nc.tensor.transpose(
                            tp[:, j * 128:(j + 1) * 128],
                            src_sb[:, col:col + 128], ident_t[:])
                    nc.vector.tensor_copy(
                        dst_t[:, g * TPG:(g + 1) * TPG, :].rearrange(
                            "p a b -> p (a b)"), tp[:])

            def dense_block(aggT_t, out_sb, bias_off, relu):
                """out_sb[128, D] = act(aggT.T @ W + b)."""
                for n in range(NT):
                    dp = dps.tile([128, ND], f32, tag="dp")
                    for k in range(KD):
                        nc.tensor.matmul(
                            dp[:], aggT_t[:, k, :], w_t[:, k, n * ND:(n + 1) * ND],
                            start=(k == 0), stop=(k == KD - 1 and not use_bias))
                    if use_bias:
                        nc.tensor.matmul(
                            dp[:], ones_t[:, :ND],
                            brow_t[:, bias_off + n * ND:bias_off + (n + 1) * ND],
                            start=False, stop=True)
                    nc.scalar.activation(out_sb[:, n * ND:(n + 1) * ND], dp[:],
                                         RELU if relu else COPY)

            ag_done = {0: set(), 1: set()}
            ag_fired = {0: set(), 1: set()}

            def stage_ag(layer, b, src_dram, dst_dram):
                """Fire any AllGather chunk whose blocks are all finished."""
                ag_done[layer].add(b)
                for a in range(len(SPL_AG) - 1):
                    blks = set(range(SPL_AG[a], SPL_AG[a + 1]))
                    if a in ag_fired[layer] or not blks <= ag_done[layer]:
                        continue
                    ag_fired[layer].add(a)
                    r0, r1 = SPL_AG[a] * 128, SPL_AG[a + 1] * 128
                    nc.gpsimd.collective_compute(
                        "AllGather", mybir.AluOpType.bypass,
                        ins=[src_dram[r0:r1, :]],
                        outs=[dst_dram[N_CORES * r0:N_CORES * r1, :]],
                        replica_groups=[list(range(N_CORES))])

            def finish_block(layer, b, agg):
                """Dense tail for a completed aggregate of layer 1/2."""
                agg_sb = wp.tile([128, D], bf16, tag="aggsb")
                nc.scalar.activation(agg_sb[:], agg[:], COPY)
                aggT_t = wp.tile([128, KD, 128], bf16, tag="aggT")
                transpose_to(aggT_t, agg_sb)
                x_sb = wp.tile([128, D], bf16, tag="x")
                dense_block(aggT_t, x_sb, layer * D, relu=True)
                if layer == 0:
                    # SWDGE write: keeps the HWDGE load rings free-flowing
                    nc.gpsimd.dma_start(ag_in[b * 128:(b + 1) * 128, :],
                                        x_sb[:])
                    stage_ag(0, b, ag_in, ag_out)
                else:
                    # y3 = x3 @ W3 (padded to CP wide) for this block
                    x3T_t = wp.tile([128, KD, 128], bf16, tag="x3T")
                    transpose_to(x3T_t, x_sb)
                    yp = dps.tile([128, CP], f32, tag="dp")
                    for k in range(KD):
                        nc.tensor.matmul(yp[:], x3T_t[:, k, :], w3_t[:, k, :],
                                         start=(k == 0), stop=(k == KD - 1))
                    y_sb = wp.tile([128, CP], bf16, tag="y")
                    nc.scalar.activation(y_sb[:], yp[:], COPY)
                    nc.sync.dma_start(ag3_in[b * 128:(b + 1) * 128, :], y_sb[:])
                    stage_ag(1, b, ag3_in, ag3_out)

            # warm-up collective: absorbs the first-call ncfw/comm init
            # latency while layer 1 computes
            nc.gpsimd.collective_compute(
                "AllGather", mybir.AluOpType.bypass,
                ins=[agw_in[:]], outs=[agw_out[:]],
                replica_groups=[list(range(N_CORES))])

            # ---------------- layer 1 (host-expanded messages, no gathers)
            # 1-block software pipeline: block b+1's scatter-matmuls are
            # queued ahead of block b's transpose/dense so the PE never
            # stalls on the PSUM->SBUF copies.
            ORD = list(range(SPL_DEP[1], NBLK)) + list(range(SPL_DEP[1]))
            prev = prev_b = None
            for b in ORD:
                load_s(b)
                agg = spmm_l1(b)
                if prev is not None:
                    finish_block(0, prev_b, prev)
                prev, prev_b = agg, b
            finish_block(0, prev_b, prev)

            nc.scalar.dma_start(w_t[:], w12_h[1])

            # ---------------- layer 2 pass A: second-half sources (their AG
            # chunk fires first) -> partial aggregates, overlapping L1's tail
            part_t = pp.tile([128, NBLK, D], bf16, tag="pA")
            for b in range(NBLK):
                agg = spmm_pass(b, 1, ag_out, D, mp, "m")
                nc.scalar.activation(part_t[:, b, :], agg[:], COPY)

            # ---------------- layer 2 pass B: first-half + partial -> dense
            prev = prev_b = None
            for b in ORD:
                agg = spmm_pass(b, 0, ag_out, D, mp, "m",
                                partial=part_t[:, b, :])
                if prev is not None:
                    finish_block(1, prev_b, prev)
                prev, prev_b = agg, b
            finish_block(1, prev_b, prev)

            # ---------------- layer 3: out = A y3 (+ b3), two passes
            part3_t = pp.tile([128, NBLK, CP], bf16, tag="pA3")
            for b in range(NBLK):
                agg = spmm_pass(b, 1, ag3_out, CP, mp3, "m3")
                nc.scalar.activation(part3_t[:, b, :], agg[:], COPY)
            for b in range(NBLK):
                agg3 = spmm_pass(b, 0, ag3_out, CP, mp3, "m3",
                                 partial=part3_t[:, b, :],
                                 close=not use_bias)
                if use_bias:
                    nc.tensor.matmul(agg3[:], ones_t[:, :CP],
                                     brow_t[:, 2 * D:2 * D + CP],
                                     start=False, stop=True)
                o_sb = wp.tile([128, C], f32, tag="o")
                nc.scalar.activation(o_sb[:], agg3[:, :C], COPY)
                nc.sync.dma_start(out_h[b * 128:(b + 1) * 128, :], o_sb[:])

    nc.compile()
    return nc


_CACHE = {}


def _get_prog(cfg, kts0, kts1, use_bias):
    key = (cfg["N"], cfg["D"], kts0, kts1, use_bias, cfg["CH"], cfg["CH1"])
    if key not in _CACHE:
        _CACHE[key] = _build(cfg, kts0, kts1, use_bias)
    return _CACHE[key]


# ---------------------------------------------------------------- entry point
CFG_FULL = dict(N=10000, E=160000, D=1024, C=64, NBLK=10, CH=4, CH1=4)


def _make_inmaps(ins, pp, cfg):
    """Per-core input dicts (host arrays in bf16)."""
    D, C, KD, CP = cfg["D"], cfg["C"], cfg["D"] // 128, 128
    h16 = np.asarray(ins["h"], np.float32).astype(BF16)
    kt_tot = pp["kt_tot"]
    w12 = np.stack([
        np.asarray(ins["W1"], np.float32).reshape(KD, 128, D).transpose(1, 0, 2),
        np.asarray(ins["W2"], np.float32).reshape(KD, 128, D).transpose(1, 0, 2),
    ]).astype(BF16)
    w3p = np.zeros((cfg["D"], CP), np.float32)
    w3p[:, :C] = np.asarray(ins["W3"], np.float32)
    w3 = w3p.reshape(KD, 128, CP).transpose(1, 0, 2).astype(BF16)
    biases = np.concatenate([
        np.asarray(ins["b1"], np.float32),
        np.asarray(ins["b2"], np.float32),
        np.asarray(ins["b3"], np.float32), np.zeros(CP - C, np.float32),
        np.ones(512, np.float32)]).astype(BF16)[None, :]
    ident = np.eye(128, dtype=BF16)
    maps = []
    for c in range(N_CORES):
        # host-side edge expansion of layer-1 messages:
        # msg1[p, kt, :] = h[src_of_slot[kt*128 + p]]
        msg1 = h16[pp["slots"][c]].reshape(kt_tot, 128, D).transpose(1, 0, 2)
        maps.append(dict(
            msg1=np.ascontiguousarray(msg1),
            sker=np.ascontiguousarray(pp["S"][c]),
            idx23=pp["idx23"][c],
            w12=w12, w3=w3, ident=ident, biases=biases))
    return maps


def kernel(h, src, dst, W1, b1, W2, b2, W3, b3, cfg=CFG_FULL):
    from concourse.bass_utils import run_bass_kernel_spmd

    h = np.asarray(h, np.float32)
    src = np.asarray(src, np.int32)
    dst = np.asarray(dst, np.int32)
    N, C = cfg["N"], cfg["C"]

    pp = _prep(h, src, dst, cfg)
    use_bias = bool(np.any(b1) or np.any(b2) or np.any(b3))
    nc = _get_prog(cfg, pp["kts0"], pp["kts1"], use_bias)

    ins = dict(h=h, W1=W1, b1=b1, W2=W2, b2=b2, W3=W3, b3=b3)
    in_maps = _make_inmaps(ins, pp, cfg)
    res = run_bass_kernel_spmd(nc, in_maps, core_ids=list(range(N_CORES)))

    out = np.zeros((N, C), np.float32)
    rows = pp["row_of_node"]
    allout = np.concatenate([res.results[c]["out"] for c in range(N_CORES)],
                            axis=0)
    out[:, :] = allout[rows]
    return out
